# revision 18
# baseline (speedup 1.0000x reference)
"""Engram block (hash-embedding gather + gated value + dilated causal depthwise
conv) as a Bass/Tile SPMD kernel on 8 Trainium2 NeuronCores.

Sharding: sequence (L) split 8 ways; each core recomputes a 12-position halo
for the causal conv. Embedding tables are replicated (the gather reads only
needed rows). Weights host-transposed/cast to bf16.

Per-core pipeline, in 3 super-iterations (one per 3 m-tiles / 1 v-chunk):
  1. indirect-DMA gather of 12 head embeddings (one batched gather per
     128-token m-tile) -> xbar DMA-transpose -> embT [e, m]. PE does no
     transposes for emb/v.
  2. K path in [tok, d] layout: stationary = embT block, moving = Wk^T cols;
     PSUM out [m_tile, 512]x4 so RMS/gate stats are free-dim reductions
     (ACT square-accumulate, DVE scalar_tensor_tensor accumulate).
  3. gate tail batched per 3 tiles on [128, 3] stats (2 ACT-table loads per
     batch instead of per tile), then gate row is PE-transposed and
     PE-broadcast ([1,128] ones stationary) into PSUM [128, 384].
  4. V path directly in [d, m] layout: stationary = Wv^T block (natural
     layout), moving = embT cols; gate applied by DVE tensor_tensor from
     two PSUM operands straight into v_sb [d, m] bf16 -- the conv layout.
  5. dilated causal conv = 4 free-dim-shifted fused multiply-adds on DVE in
     bf16; out stored bf16 as [D, m_out] in two half-D DMAs per range (host
     re-transposes and casts when unsharding).
"""
import sys

sys.path.insert(0, "/opt/trn_rl_repo")

import numpy as np
import ml_dtypes

import concourse.bass as bass
import concourse.tile as tile
from concourse import mybir
from concourse.masks import make_identity
from concourse.bass_utils import run_bass_kernel_spmd

# problem shapes (hardcoded per spec)
L, B, D = 4096, 2, 2048
H, Dh = 12, 128
E = H * Dh  # 1536
N = 100000
K, DIL = 4, 4
EPS = 1e-6

NCORES = 8
LC = L // NCORES          # 512 l-positions per core
HALO = (K - 1) * DIL      # 12
LE = LC + HALO            # 524
M = LE * B                # 1048 valid tokens (l-major, b inner)
MP = 1152                 # padded to 9*128
MT = MP // 128            # 9 m-tiles
DT = D // 128             # 16 d-tiles
ET = E // 128             # 12 e-tiles
MOUT = LC * B             # 1024 output tokens per core
OFF = HALO * B            # 24 = first valid output token
D2 = 2 * D                # concat k|v weight cols
KG = 4                    # K-path psum groups
KW = D // KG              # 512 cols per K group
NB = 3                    # super-iterations (batches)
TPB = MT // NB            # 3 m-tiles per batch
CW = [384, 384, 280]      # V-chunk widths (m cols)
C0 = [0, 384, 768]        # V-chunk start col
# conv out-col ranges; range c reads v cols <= C0[c]+CW[c]
CONV_R = [(0, 360), (360, 384), (744, 280)]

BF16 = mybir.dt.bfloat16
F32 = mybir.dt.float32
I32 = mybir.dt.int32

# scal columns per d-tile
SC_W0, SC_W1, SC_W2, SC_W3P, SC_CB = range(5)
NSC = 5


def _split_multi_waits(nc):
    """This walrus build accepts only one sync-wait per instruction; hoist
    extra waits onto injected NOPs on the same engine (order-preserving)."""
    for f in nc.m.functions:
        for bb in f.blocks:
            new_insts = []
            for inst in bb.instructions:
                si = inst.sync_info
                if si is not None and si.on_wait and len(si.on_wait) > 1:
                    for w in si.on_wait[:-1]:
                        nop = mybir.InstNoOp(
                            name=nc.get_next_instruction_name(), ins=[], outs=[]
                        )
                        nop.engine = inst.engine
                        nop.sync_info = mybir.SyncInfo(on_wait=[w], on_update=[])
                        new_insts.append(nop)
                    si.on_wait = [si.on_wait[-1]]
                new_insts.append(inst)
            bb.instructions = new_insts


def build_program():
    nc = bass.Bass("TRN2", target_bir_lowering=False, debug=False)

    tabs = nc.declare_dram_parameter("tabs", [H * N, Dh], BF16, isOutput=False)
    ids = nc.declare_dram_parameter("ids", [128, MT * H], I32, isOutput=False)
    hid = nc.declare_dram_parameter("hid", [MP, D], BF16, isOutput=False)
    wkv = nc.declare_dram_parameter("wkv", [E, D2], BF16, isOutput=False)
    scal = nc.declare_dram_parameter("scal", [128, DT * NSC], F32, isOutput=False)
    outT = nc.declare_dram_parameter("outT", [D, MOUT], BF16, isOutput=True)

    AR = mybir.ActivationFunctionType
    ALU = mybir.AluOpType

    with tile.TileContext(nc) as tc:
        with (
            tc.tile_pool(name="persist", bufs=1) as pp,
            tc.tile_pool(name="work", bufs=2) as wp,
            tc.tile_pool(name="stat", bufs=2) as sp,
            tc.tile_pool(name="psum", bufs=1, space="PSUM") as psp,
        ):
            # ---- constants / small inputs ----
            eps_sb = pp.tile([128, 1], F32, tag="eps")
            nc.vector.memset(eps_sb[:], EPS)
            ones3 = pp.tile([TPB, 128], F32, tag="ones3")
            nc.vector.memset(ones3[:], 1.0)
            identf = pp.tile([128, 128], F32, tag="identf")
            make_identity(nc, identf[:])

            ids_sb = pp.tile([128, MT * H], I32, tag="ids")
            nc.sync.dma_start(ids_sb[:], ids.ap())
            scal_sb = pp.tile([128, DT * NSC], F32, tag="scal")
            nc.sync.dma_start(scal_sb[:], scal.ap())

            def sc(dt_, c):
                return scal_sb[:, dt_ * NSC + c : dt_ * NSC + c + 1]

            # ---- weights (resident, concat k|v along cols) ----
            wkv_sb = []
            for e in range(ET):
                w = pp.tile([128, D2], BF16, tag=f"wkv{e}", name=f"wkv{e}")
                nc.scalar.dma_start(w[:], wkv[e * 128 : (e + 1) * 128, :])
                wkv_sb.append(w)

            # ---- gather all m-tiles up front (program order sets priority;
            #      SWDGE streams ahead of PE consumption). One batched
            #      12-head indirect gather per m-tile. ----
            bc_reg = nc.gpsimd.to_reg(H * N - 1)
            emb_raws = []
            for t in range(MT):
                er = wp.tile(
                    [128, H, Dh], BF16, tag="emb_raw", bufs=3, name=f"er{t}"
                )
                if t in (0, MT - 1):
                    nc.gpsimd.memset(er[:], 0)
                for h in range(H):
                    nc.gpsimd.indirect_dma_start(
                        out=er[:, h, :],
                        out_offset=None,
                        in_=tabs[:],
                        in_offset=bass.IndirectOffsetOnAxis(
                            ap=ids_sb[:, t * H + h : t * H + h + 1], axis=0
                        ),
                        bounds_check=bc_reg,
                        oob_is_err=False,
                    )
                emb_raws.append(er)

            embT = [
                pp.tile([128, MP], BF16, tag=f"embT{h}", name=f"embT{h}")
                for h in range(H)
            ]
            v_sb = [
                pp.tile([128, MP], BF16, tag=f"v_sb{dt_}", name=f"v_sb{dt_}")
                for dt_ in range(DT)
            ]

            for c in range(NB):
                # ---------- K path: 3 m-tiles ----------
                sk_all = sp.tile([128, TPB * KG], F32, tag="sk_all")
                pk_all = sp.tile([128, TPB * KG], F32, tag="pk_all")
                sh_all = sp.tile([128, TPB], F32, tag="sh_all")
                for tl in range(TPB):
                    t = c * TPB + tl
                    er = emb_raws[t]
                    # xbar-transpose 12 head blocks -> embT (sync HWDGE)
                    for h in range(H):
                        nc.sync.dma_start(
                            embT[h][:, t * 128 : (t + 1) * 128],
                            er[:, h, :],
                            transpose=True,
                        )

                    # hidden rows for this m-tile + h^2 accum
                    h_md = wp.tile([128, D], BF16, tag="h_md", bufs=2)
                    nc.scalar.dma_start(
                        h_md[:], hid.ap()[t * 128 : (t + 1) * 128, :]
                    )
                    jh = wp.tile([128, D], BF16, tag="jh", bufs=1, name="jh")
                    nc.scalar.activation(
                        out=jh[:], in_=h_md[:], func=AR.Square,
                        accum_out=sh_all[:, tl : tl + 1],
                    )

                    # k matmuls: 4 groups of 512 cols, e-contraction inner
                    for g in range(KG):
                        kps = psp.tile(
                            [128, KW], F32, tag="kps", bufs=4, space="PSUM"
                        )
                        for e in range(ET):
                            nc.tensor.matmul(
                                out=kps[:],
                                lhsT=embT[e][:, t * 128 : (t + 1) * 128],
                                rhs=wkv_sb[e][:, g * KW : (g + 1) * KW],
                                start=(e == 0), stop=(e == ET - 1),
                            )
                        jk = wp.tile([128, KW], BF16, tag="jk", bufs=2, name="jk")
                        nc.scalar.activation(
                            out=jk[:], in_=kps[:], func=AR.Square,
                            accum_out=sk_all[:, tl * KG + g : tl * KG + g + 1],
                        )
                        jkh = wp.tile([128, KW], BF16, tag="jkh", bufs=2, name="jkh")
                        nc.vector.scalar_tensor_tensor(
                            out=jkh[:], in0=kps[:], scalar=1.0,
                            in1=h_md[:, g * KW : (g + 1) * KW],
                            op0=ALU.mult, op1=ALU.mult,
                            accum_out=pk_all[:, tl * KG + g : tl * KG + g + 1],
                        )

                # ---------- batched gate tail on [128, TPB] ----------
                sk_r = sp.tile([128, TPB], F32, tag="sk_r")
                nc.vector.reduce_sum(
                    out=sk_r[:],
                    in_=sk_all[:].rearrange("p (t g) -> p t g", g=KG),
                    axis=mybir.AxisListType.X,
                )
                pk_r = sp.tile([128, TPB], F32, tag="pk_r")
                nc.vector.reduce_sum(
                    out=pk_r[:],
                    in_=pk_all[:].rearrange("p (t g) -> p t g", g=KG),
                    axis=mybir.AxisListType.X,
                )
                a_ = sp.tile([128, TPB], F32, tag="a_")
                nc.scalar.activation(
                    out=a_[:], in_=sk_r[:], func=AR.Identity,
                    bias=eps_sb[:, 0:1], scale=1.0 / D,
                )
                b_ = sp.tile([128, TPB], F32, tag="b_")
                nc.scalar.activation(
                    out=b_[:], in_=sh_all[:], func=AR.Identity,
                    bias=eps_sb[:, 0:1], scale=1.0 / D,
                )
                tt = sp.tile([128, TPB], F32, tag="tt")
                nc.vector.tensor_mul(tt[:], a_[:], b_[:])
                rr = sp.tile([128, TPB], F32, tag="rr")
                nc.vector.reciprocal(rr[:], tt[:])
                rq = sp.tile([128, TPB], F32, tag="rq")
                nc.scalar.activation(out=rq[:], in_=rr[:], func=AR.Sqrt)
                uu = sp.tile([128, TPB], F32, tag="uu")
                nc.vector.scalar_tensor_tensor(
                    out=uu[:], in0=pk_r[:], scalar=float(1.0 / np.sqrt(D)),
                    in1=rq[:], op0=ALU.mult, op1=ALU.mult,
                )
                ab = sp.tile([128, TPB], F32, tag="ab")
                nc.scalar.activation(out=ab[:], in_=uu[:], func=AR.Abs)
                mx = sp.tile([128, TPB], F32, tag="mx")
                nc.vector.tensor_scalar_max(out=mx[:], in0=ab[:], scalar1=1e-6)
                r2 = sp.tile([128, TPB], F32, tag="r2")
                nc.vector.reciprocal(r2[:], mx[:])
                q2 = sp.tile([128, TPB], F32, tag="q2")
                nc.scalar.activation(out=q2[:], in_=r2[:], func=AR.Sqrt)
                st = sp.tile([128, TPB], F32, tag="st")
                nc.vector.tensor_mul(st[:], uu[:], q2[:])
                g3 = sp.tile([128, TPB], F32, tag="g3")
                nc.scalar.activation(out=g3[:], in_=st[:], func=AR.Sigmoid)

                # gate row: PE-transpose each [128,1] gate column to a
                # [1,128] row segment (all at partition 0), copy to SBUF
                gt_ps = psp.tile([128, 384], F32, tag="gt", bufs=1, space="PSUM")
                for j in range(TPB):
                    nc.tensor.transpose(
                        out=gt_ps[0:1, j * 128 : (j + 1) * 128],
                        in_=g3[:, j : j + 1], identity=identf[:],
                    )
                gt_sb = sp.tile([1, 384], F32, tag="gt_sb", bufs=1)
                nc.scalar.copy(out=gt_sb[:], in_=gt_ps[0:1, :])

                # ---------- V path: chunk c in [d, m] layout ----------
                w_c = CW[c]
                c0o, cwo = CONV_R[c]
                bc_ps = psp.tile([128, 384], F32, tag="bc", bufs=1, space="PSUM")
                bc_sb = sp.tile([128, 384], BF16, tag="bc_sb", bufs=1)
                ot_cur = None
                for db in range(DT):
                    vps = psp.tile([128, 384], F32, tag="vps", bufs=2, space="PSUM")
                    for e in range(ET):
                        nc.tensor.matmul(
                            out=vps[:, :w_c],
                            lhsT=wkv_sb[e][:, D + db * 128 : D + (db + 1) * 128],
                            rhs=embT[e][:, C0[c] : C0[c] + w_c],
                            start=(e == 0), stop=(e == ET - 1),
                        )
                    if db == 0:
                        # gate broadcast: [1,128] ones stationary x gate row
                        # (emitted after db0's matmuls so PE stays busy while
                        # the tail chain resolves)
                        for j in range(TPB):
                            nc.tensor.matmul(
                                out=bc_ps[:, j * 128 : (j + 1) * 128],
                                lhsT=ones3[0:1, :],
                                rhs=gt_sb[0:1, j * 128 : (j + 1) * 128],
                                start=True, stop=True,
                            )
                        # DVE can read only one PSUM operand per op: stage
                        # the broadcast in SBUF for the 16 gate-multiplies
                        nc.scalar.copy(out=bc_sb[:], in_=bc_ps[:])
                    # gate-multiply into conv layout (bf16)
                    nc.vector.tensor_mul(
                        v_sb[db][:, C0[c] : C0[c] + w_c],
                        vps[:, :w_c],
                        bc_sb[:, :w_c],
                    )
                    # conv for this range + d-block (DVE, bf16)
                    a1 = wp.tile([128, 384], BF16, tag="a1", bufs=2)
                    nc.vector.tensor_scalar(
                        out=a1[:, :cwo], in0=v_sb[db][:, c0o : c0o + cwo],
                        scalar1=sc(db, SC_W0), scalar2=sc(db, SC_CB),
                        op0=ALU.mult, op1=ALU.add,
                    )
                    a2 = wp.tile([128, 384], BF16, tag="a2", bufs=2)
                    nc.vector.scalar_tensor_tensor(
                        out=a2[:, :cwo], in0=v_sb[db][:, c0o + 8 : c0o + 8 + cwo],
                        scalar=sc(db, SC_W1), in1=a1[:, :cwo],
                        op0=ALU.mult, op1=ALU.add,
                    )
                    a3 = wp.tile([128, 384], BF16, tag="a3", bufs=2)
                    nc.vector.scalar_tensor_tensor(
                        out=a3[:, :cwo], in0=v_sb[db][:, c0o + 16 : c0o + 16 + cwo],
                        scalar=sc(db, SC_W2), in1=a2[:, :cwo],
                        op0=ALU.mult, op1=ALU.add,
                    )
                    half = db // 8
                    if db % 8 == 0:
                        ot_cur = wp.tile(
                            [128, 8, 384], BF16, tag="ot", bufs=2,
                            name=f"ot{c}_{half}",
                        )
                    nc.vector.scalar_tensor_tensor(
                        out=ot_cur[:, db % 8, :cwo],
                        in0=v_sb[db][:, c0o + OFF : c0o + OFF + cwo],
                        scalar=sc(db, SC_W3P), in1=a3[:, :cwo],
                        op0=ALU.mult, op1=ALU.add,
                    )
                    if db % 8 == 7:
                        nc.sync.dma_start(
                            outT.ap()
                            .rearrange("(dt p) m -> p dt m", p=128)[
                                :, half * 8 : (half + 1) * 8, c0o : c0o + cwo
                            ],
                            ot_cur[:, :, :cwo],
                        )

    _split_multi_waits(nc)
    return nc


_CACHE = {}


def _get_program():
    if "nc" not in _CACHE:
        _CACHE["nc"] = build_program()
    return _CACHE["nc"]


def host_prep(hidden_states, hash_input_ids, emb_tables, key_w, key_b,
              norm1_w, norm2_w, value_w, value_b, conv_w, conv_b):
    """Shard + lay out inputs for the 8 cores. Returns in_maps list."""
    bf = ml_dtypes.bfloat16
    w12 = norm1_w.astype(np.float64) * norm2_w.astype(np.float64)
    assert np.allclose(w12, 1.0, atol=1e-5), (
        "fast path assumes norm1_w*norm2_w == 1 (problem spec: fill=ones)"
    )
    assert not key_b.any() and not value_b.any(), (
        "fast path assumes zero key/value biases (problem spec: fill=zeros)"
    )

    tabs_np = np.ascontiguousarray(emb_tables.reshape(H * N, Dh)).astype(bf)
    wkv_np = np.empty((E, D2), bf)
    wkv_np[:, :D] = key_w.T.astype(bf)
    wkv_np[:, D:] = value_w.T.astype(bf)
    scal_d = np.empty((D, NSC), np.float32)
    scal_d[:, SC_W0] = conv_w[:, 0]
    scal_d[:, SC_W1] = conv_w[:, 1]
    scal_d[:, SC_W2] = conv_w[:, 2]
    scal_d[:, SC_W3P] = conv_w[:, 3] + 1.0
    scal_d[:, SC_CB] = conv_b
    scal_np = np.ascontiguousarray(
        scal_d.reshape(DT, 128, NSC).transpose(1, 0, 2).reshape(128, DT * NSC)
    )

    head_off = (np.arange(H, dtype=np.int64) * N)[None, :]
    OOB = np.int32(H * N)

    in_maps = []
    for c in range(NCORES):
        l0 = c * LC
        lo = l0 - HALO
        lo_clip = max(lo, 0)
        nvalid = (l0 + LC) - lo_clip
        r0 = (lo_clip - lo) * B
        ids_c = np.full((MP, H), OOB, np.int32)
        seg = hash_input_ids[lo_clip : l0 + LC].reshape(nvalid * B, H)
        ids_c[r0 : r0 + nvalid * B] = (seg.astype(np.int64) + head_off).astype(
            np.int32
        )
        hid_c = np.zeros((MP, D), bf)
        hseg = hidden_states[lo_clip : l0 + LC].reshape(nvalid * B, D)
        hid_c[r0 : r0 + nvalid * B] = hseg.astype(bf)
        ids_r = np.ascontiguousarray(
            ids_c.reshape(MT, 128, H).transpose(1, 0, 2).reshape(128, MT * H)
        )
        in_maps.append(
            {
                "tabs": tabs_np,
                "ids": ids_r,
                "hid": hid_c,
                "wkv": wkv_np,
                "scal": scal_np,
            }
        )
    return in_maps


def unshard_output(results):
    """results: list of per-core dicts with 'outT' [D, MOUT] -> [L, B, D]."""
    out = np.empty((L, B, D), np.float32)
    for c in range(NCORES):
        o = np.asarray(results[c]["outT"], dtype=np.float32)
        out[c * LC : (c + 1) * LC] = o.reshape(D, LC, B).transpose(1, 2, 0)
    return out


def kernel(hidden_states, hash_input_ids, emb_tables, key_w, key_b,
           norm1_w, norm2_w, value_w, value_b, conv_w, conv_b):
    args = [hidden_states, hash_input_ids, emb_tables, key_w, key_b,
            norm1_w, norm2_w, value_w, value_b, conv_w, conv_b]
    args = [np.asarray(a) for a in args]
    in_maps = host_prep(*args)
    nc = _get_program()
    res = run_bass_kernel_spmd(nc, in_maps, list(range(NCORES)))
    return unshard_output(res.results)


# revision 22
# speedup vs baseline: 2.5726x; 2.5726x over previous
"""Engram block (hash-embedding gather + gated value + dilated causal depthwise
conv) as a Bass/Tile SPMD kernel on 8 Trainium2 NeuronCores.

Sharding: sequence (L) split 8 ways; each core recomputes a 12-position halo
for the causal conv. Embedding tables are replicated (the gather reads only
needed rows). Weights host-transposed/cast to bf16.

Per-core pipeline, in 3 super-iterations (one per 3 m-tiles / 1 v-chunk):
  1. indirect-DMA gather of 12 head embeddings (one batched gather per
     128-token m-tile) -> xbar DMA-transpose -> embT [e, m]. PE does no
     transposes for emb/v.
  2. K path in [tok, d] layout: stationary = embT block, moving = Wk^T cols;
     PSUM out [m_tile, 512]x4 so RMS/gate stats are free-dim reductions
     (ACT square-accumulate, DVE scalar_tensor_tensor accumulate).
  3. gate tail batched per 3 tiles on [128, 3] stats (2 ACT-table loads per
     batch instead of per tile), then gate row is PE-transposed and
     PE-broadcast ([1,128] ones stationary) into PSUM [128, 384].
  4. V path directly in [d, m] layout: stationary = Wv^T block (natural
     layout), moving = embT cols; gate applied by DVE tensor_tensor from
     two PSUM operands straight into v_sb [d, m] bf16 -- the conv layout.
  5. dilated causal conv = 4 free-dim-shifted fused multiply-adds on DVE in
     bf16; out stored bf16 as [D, m_out] in two half-D DMAs per range (host
     re-transposes and casts when unsharding).
"""
import sys

sys.path.insert(0, "/opt/trn_rl_repo")

import numpy as np
import ml_dtypes

import concourse.bass as bass
import concourse.tile as tile
from concourse import mybir
from concourse.masks import make_identity
from concourse.bass_utils import run_bass_kernel_spmd

# problem shapes (hardcoded per spec)
L, B, D = 4096, 2, 2048
H, Dh = 12, 128
E = H * Dh  # 1536
N = 100000
K, DIL = 4, 4
EPS = 1e-6

NCORES = 8
LC = L // NCORES          # 512 l-positions per core
HALO = (K - 1) * DIL      # 12
LE = LC + HALO            # 524
M = LE * B                # 1048 valid tokens (l-major, b inner)
MP = 1152                 # padded to 9*128
MT = MP // 128            # 9 m-tiles
DT = D // 128             # 16 d-tiles
ET = E // 128             # 12 e-tiles
MOUT = LC * B             # 1024 output tokens per core
OFF = HALO * B            # 24 = first valid output token
D2 = 2 * D                # concat k|v weight cols
KG = 4                    # K-path psum groups
KW = D // KG              # 512 cols per K group
NB = 3                    # super-iterations (batches)
TPB = MT // NB            # 3 m-tiles per batch
CW = [384, 384, 280]      # V-chunk widths (m cols)
C0 = [0, 384, 768]        # V-chunk start col
# conv out-col ranges; range c reads v cols <= C0[c]+CW[c]
CONV_R = [(0, 360), (360, 384), (744, 280)]

BF16 = mybir.dt.bfloat16
F32 = mybir.dt.float32
I32 = mybir.dt.int32

# scal columns per d-tile
SC_W0, SC_W1, SC_W2, SC_W3P, SC_CB = range(5)
NSC = 5


def _split_multi_waits(nc):
    """This walrus build accepts only one sync-wait per instruction; hoist
    extra waits onto injected NOPs on the same engine (order-preserving)."""
    for f in nc.m.functions:
        for bb in f.blocks:
            new_insts = []
            for inst in bb.instructions:
                si = inst.sync_info
                if si is not None and si.on_wait and len(si.on_wait) > 1:
                    for w in si.on_wait[:-1]:
                        nop = mybir.InstNoOp(
                            name=nc.get_next_instruction_name(), ins=[], outs=[]
                        )
                        nop.engine = inst.engine
                        nop.sync_info = mybir.SyncInfo(on_wait=[w], on_update=[])
                        new_insts.append(nop)
                    si.on_wait = [si.on_wait[-1]]
                new_insts.append(inst)
            bb.instructions = new_insts


def build_program():
    nc = bass.Bass("TRN2", target_bir_lowering=False, debug=False)

    tabs = nc.declare_dram_parameter("tabs", [H * N, Dh], BF16, isOutput=False)
    ids = nc.declare_dram_parameter("ids", [128, MT * H], I32, isOutput=False)
    hid = nc.declare_dram_parameter("hid", [MP, D], BF16, isOutput=False)
    wkv = nc.declare_dram_parameter("wkv", [E, D2], BF16, isOutput=False)
    scal = nc.declare_dram_parameter("scal", [128, DT * NSC], F32, isOutput=False)
    outT = nc.declare_dram_parameter("outT", [D, MOUT], BF16, isOutput=True)

    AR = mybir.ActivationFunctionType
    ALU = mybir.AluOpType

    with tile.TileContext(nc) as tc:
        with (
            tc.tile_pool(name="persist", bufs=1) as pp,
            tc.tile_pool(name="work", bufs=2) as wp,
            tc.tile_pool(name="stat", bufs=2) as sp,
            tc.tile_pool(name="psum", bufs=1, space="PSUM") as psp,
        ):
            # ---- constants / small inputs ----
            eps_sb = pp.tile([128, 1], F32, tag="eps")
            nc.vector.memset(eps_sb[:], EPS)
            ones3 = pp.tile([1, 128], BF16, tag="ones3")
            nc.vector.memset(ones3[:], 1.0)
            identb = pp.tile([128, 128], BF16, tag="identb")
            make_identity(nc, identb[:])

            ids_sb = pp.tile([128, MT * H], I32, tag="ids")
            nc.sync.dma_start(ids_sb[:], ids.ap())
            scal_sb = pp.tile([128, DT * NSC], F32, tag="scal")
            nc.sync.dma_start(scal_sb[:], scal.ap())

            def sc(dt_, c):
                return scal_sb[:, dt_ * NSC + c : dt_ * NSC + c + 1]

            # ---- weights (resident, concat k|v along cols) ----
            wkv_sb = []
            for e in range(ET):
                w = pp.tile([128, D2], BF16, tag=f"wkv{e}", name=f"wkv{e}")
                nc.scalar.dma_start(w[:], wkv[e * 128 : (e + 1) * 128, :])
                wkv_sb.append(w)

            # ---- gather all m-tiles up front (program order sets priority;
            #      SWDGE streams ahead of PE consumption). One batched
            #      12-head indirect gather per m-tile. ----
            bc_reg = nc.gpsimd.to_reg(H * N - 1)
            emb_raws = []
            for t in range(MT):
                er = wp.tile(
                    [128, H, Dh], BF16, tag="emb_raw", bufs=3, name=f"er{t}"
                )
                if t in (0, MT - 1):
                    nc.gpsimd.memset(er[:], 0)
                for h in range(H):
                    nc.gpsimd.indirect_dma_start(
                        out=er[:, h, :],
                        out_offset=None,
                        in_=tabs[:],
                        in_offset=bass.IndirectOffsetOnAxis(
                            ap=ids_sb[:, t * H + h : t * H + h + 1], axis=0
                        ),
                        bounds_check=bc_reg,
                        oob_is_err=False,
                    )
                emb_raws.append(er)

            embT = [
                pp.tile([128, MP], BF16, tag=f"embT{h}", name=f"embT{h}")
                for h in range(H)
            ]
            v_sb = [
                pp.tile([128, MP], BF16, tag=f"v_sb{dt_}", name=f"v_sb{dt_}")
                for dt_ in range(DT)
            ]

            for c in range(NB):
                # ---------- K path: 3 m-tiles ----------
                sk_all = sp.tile([128, TPB * KG], F32, tag="sk_all")
                pk_all = sp.tile([128, TPB * KG], F32, tag="pk_all")
                sh_all = sp.tile([128, TPB], F32, tag="sh_all")
                for tl in range(TPB):
                    t = c * TPB + tl
                    er = emb_raws[t]
                    # PE-transpose 12 head blocks -> embT (xbar DMA transpose
                    # serializes 1:1 against the SWDGE gather stream, so the
                    # PE path wins despite costing ~300ns each)
                    for h in range(H):
                        tp = psp.tile(
                            [128, 128], BF16, tag="tp", bufs=2, space="PSUM"
                        )
                        nc.tensor.transpose(
                            out=tp[:], in_=er[:, h, :], identity=identb[:]
                        )
                        nc.scalar.copy(
                            out=embT[h][:, t * 128 : (t + 1) * 128], in_=tp[:]
                        )

                    # hidden rows for this m-tile + h^2 accum
                    h_md = wp.tile([128, D], BF16, tag="h_md", bufs=2)
                    nc.scalar.dma_start(
                        h_md[:], hid.ap()[t * 128 : (t + 1) * 128, :]
                    )
                    jh = wp.tile([128, D], BF16, tag="jh", bufs=1, name="jh")
                    nc.scalar.activation(
                        out=jh[:], in_=h_md[:], func=AR.Square,
                        accum_out=sh_all[:, tl : tl + 1],
                    )

                    # k matmuls: 4 groups of 512 cols, e-contraction inner
                    for g in range(KG):
                        kps = psp.tile(
                            [128, KW], F32, tag="kps", bufs=3, space="PSUM"
                        )
                        for e in range(ET):
                            nc.tensor.matmul(
                                out=kps[:],
                                lhsT=embT[e][:, t * 128 : (t + 1) * 128],
                                rhs=wkv_sb[e][:, g * KW : (g + 1) * KW],
                                start=(e == 0), stop=(e == ET - 1),
                            )
                        jk = wp.tile([128, KW], BF16, tag="jk", bufs=2, name="jk")
                        nc.scalar.activation(
                            out=jk[:], in_=kps[:], func=AR.Square,
                            accum_out=sk_all[:, tl * KG + g : tl * KG + g + 1],
                        )
                        jkh = wp.tile([128, KW], BF16, tag="jkh", bufs=2, name="jkh")
                        nc.vector.scalar_tensor_tensor(
                            out=jkh[:], in0=kps[:], scalar=1.0,
                            in1=h_md[:, g * KW : (g + 1) * KW],
                            op0=ALU.mult, op1=ALU.mult,
                            accum_out=pk_all[:, tl * KG + g : tl * KG + g + 1],
                        )

                # ---------- batched gate tail on [128, TPB] ----------
                sk_r = sp.tile([128, TPB], F32, tag="sk_r")
                nc.vector.reduce_sum(
                    out=sk_r[:],
                    in_=sk_all[:].rearrange("p (t g) -> p t g", g=KG),
                    axis=mybir.AxisListType.X,
                )
                pk_r = sp.tile([128, TPB], F32, tag="pk_r")
                nc.vector.reduce_sum(
                    out=pk_r[:],
                    in_=pk_all[:].rearrange("p (t g) -> p t g", g=KG),
                    axis=mybir.AxisListType.X,
                )
                a_ = sp.tile([128, TPB], F32, tag="a_")
                nc.scalar.activation(
                    out=a_[:], in_=sk_r[:], func=AR.Identity,
                    bias=eps_sb[:, 0:1], scale=1.0 / D,
                )
                b_ = sp.tile([128, TPB], F32, tag="b_")
                nc.scalar.activation(
                    out=b_[:], in_=sh_all[:], func=AR.Identity,
                    bias=eps_sb[:, 0:1], scale=1.0 / D,
                )
                tt = sp.tile([128, TPB], F32, tag="tt")
                nc.vector.tensor_mul(tt[:], a_[:], b_[:])
                rr = sp.tile([128, TPB], F32, tag="rr")
                nc.vector.reciprocal(rr[:], tt[:])
                rq = sp.tile([128, TPB], F32, tag="rq")
                nc.scalar.activation(out=rq[:], in_=rr[:], func=AR.Sqrt)
                uu = sp.tile([128, TPB], F32, tag="uu")
                nc.vector.scalar_tensor_tensor(
                    out=uu[:], in0=pk_r[:], scalar=float(1.0 / np.sqrt(D)),
                    in1=rq[:], op0=ALU.mult, op1=ALU.mult,
                )
                ab = sp.tile([128, TPB], F32, tag="ab")
                nc.scalar.activation(out=ab[:], in_=uu[:], func=AR.Abs)
                mx = sp.tile([128, TPB], F32, tag="mx")
                nc.vector.tensor_scalar_max(out=mx[:], in0=ab[:], scalar1=1e-6)
                r2 = sp.tile([128, TPB], F32, tag="r2")
                nc.vector.reciprocal(r2[:], mx[:])
                q2 = sp.tile([128, TPB], F32, tag="q2")
                nc.scalar.activation(out=q2[:], in_=r2[:], func=AR.Sqrt)
                st = sp.tile([128, TPB], F32, tag="st")
                nc.vector.tensor_mul(st[:], uu[:], q2[:])
                g3 = sp.tile([128, TPB], BF16, tag="g3")
                nc.scalar.activation(out=g3[:], in_=st[:], func=AR.Sigmoid)

                # gate row: PE-transpose each [128,1] gate column to a
                # [1,128] row segment (all at partition 0), copy to SBUF
                gt_sb = sp.tile([1, 384], BF16, tag="gt_sb", bufs=1)
                for j in range(TPB):
                    tp = psp.tile(
                        [128, 128], BF16, tag="tp", bufs=2, space="PSUM"
                    )
                    nc.tensor.transpose(
                        out=tp[0:1, :], in_=g3[:, j : j + 1], identity=identb[:]
                    )
                    nc.scalar.copy(
                        out=gt_sb[0:1, j * 128 : (j + 1) * 128], in_=tp[0:1, :]
                    )

                # ---------- V path: chunk c in [d, m] layout ----------
                w_c = CW[c]
                c0o, cwo = CONV_R[c]
                bc_ps = psp.tile([128, 384], F32, tag="bc", bufs=1, space="PSUM")
                bc_sb = sp.tile([128, 384], BF16, tag="bc_sb", bufs=1)
                ot_cur = None
                for db in range(DT):
                    vps = psp.tile([128, 384], F32, tag="vps", bufs=2, space="PSUM")
                    for e in range(ET):
                        nc.tensor.matmul(
                            out=vps[:, :w_c],
                            lhsT=wkv_sb[e][:, D + db * 128 : D + (db + 1) * 128],
                            rhs=embT[e][:, C0[c] : C0[c] + w_c],
                            start=(e == 0), stop=(e == ET - 1),
                        )
                    if db == 0:
                        # gate broadcast: [1,128] ones stationary x gate row
                        # (emitted after db0's matmuls so PE stays busy while
                        # the tail chain resolves)
                        for j in range(TPB):
                            nc.tensor.matmul(
                                out=bc_ps[:, j * 128 : (j + 1) * 128],
                                lhsT=ones3[0:1, :],
                                rhs=gt_sb[0:1, j * 128 : (j + 1) * 128],
                                start=True, stop=True,
                            )
                        # DVE can read only one PSUM operand per op: stage
                        # the broadcast in SBUF for the 16 gate-multiplies
                        nc.scalar.copy(out=bc_sb[:], in_=bc_ps[:])
                    # gate-multiply into conv layout (bf16)
                    nc.vector.tensor_mul(
                        v_sb[db][:, C0[c] : C0[c] + w_c],
                        vps[:, :w_c],
                        bc_sb[:, :w_c],
                    )
                    # conv for this range + d-block (DVE, bf16)
                    a1 = wp.tile([128, 384], BF16, tag="a1", bufs=2)
                    nc.vector.tensor_scalar(
                        out=a1[:, :cwo], in0=v_sb[db][:, c0o : c0o + cwo],
                        scalar1=sc(db, SC_W0), scalar2=sc(db, SC_CB),
                        op0=ALU.mult, op1=ALU.add,
                    )
                    a2 = wp.tile([128, 384], BF16, tag="a2", bufs=2)
                    nc.vector.scalar_tensor_tensor(
                        out=a2[:, :cwo], in0=v_sb[db][:, c0o + 8 : c0o + 8 + cwo],
                        scalar=sc(db, SC_W1), in1=a1[:, :cwo],
                        op0=ALU.mult, op1=ALU.add,
                    )
                    a3 = wp.tile([128, 384], BF16, tag="a3", bufs=2)
                    nc.vector.scalar_tensor_tensor(
                        out=a3[:, :cwo], in0=v_sb[db][:, c0o + 16 : c0o + 16 + cwo],
                        scalar=sc(db, SC_W2), in1=a2[:, :cwo],
                        op0=ALU.mult, op1=ALU.add,
                    )
                    half = db // 8
                    if db % 8 == 0:
                        ot_cur = wp.tile(
                            [128, 8, 384], BF16, tag="ot", bufs=2,
                            name=f"ot{c}_{half}",
                        )
                    nc.vector.scalar_tensor_tensor(
                        out=ot_cur[:, db % 8, :cwo],
                        in0=v_sb[db][:, c0o + OFF : c0o + OFF + cwo],
                        scalar=sc(db, SC_W3P), in1=a3[:, :cwo],
                        op0=ALU.mult, op1=ALU.add,
                    )
                    if db % 8 == 7:
                        nc.sync.dma_start(
                            outT.ap()
                            .rearrange("(dt p) m -> p dt m", p=128)[
                                :, half * 8 : (half + 1) * 8, c0o : c0o + cwo
                            ],
                            ot_cur[:, :, :cwo],
                        )

    _split_multi_waits(nc)
    return nc


_CACHE = {}


def _get_program():
    if "nc" not in _CACHE:
        _CACHE["nc"] = build_program()
    return _CACHE["nc"]


def host_prep(hidden_states, hash_input_ids, emb_tables, key_w, key_b,
              norm1_w, norm2_w, value_w, value_b, conv_w, conv_b):
    """Shard + lay out inputs for the 8 cores. Returns in_maps list."""
    bf = ml_dtypes.bfloat16
    w12 = norm1_w.astype(np.float64) * norm2_w.astype(np.float64)
    assert np.allclose(w12, 1.0, atol=1e-5), (
        "fast path assumes norm1_w*norm2_w == 1 (problem spec: fill=ones)"
    )
    assert not key_b.any() and not value_b.any(), (
        "fast path assumes zero key/value biases (problem spec: fill=zeros)"
    )

    tabs_np = np.ascontiguousarray(emb_tables.reshape(H * N, Dh)).astype(bf)
    wkv_np = np.empty((E, D2), bf)
    wkv_np[:, :D] = key_w.T.astype(bf)
    wkv_np[:, D:] = value_w.T.astype(bf)
    scal_d = np.empty((D, NSC), np.float32)
    scal_d[:, SC_W0] = conv_w[:, 0]
    scal_d[:, SC_W1] = conv_w[:, 1]
    scal_d[:, SC_W2] = conv_w[:, 2]
    scal_d[:, SC_W3P] = conv_w[:, 3] + 1.0
    scal_d[:, SC_CB] = conv_b
    scal_np = np.ascontiguousarray(
        scal_d.reshape(DT, 128, NSC).transpose(1, 0, 2).reshape(128, DT * NSC)
    )

    head_off = (np.arange(H, dtype=np.int64) * N)[None, :]
    OOB = np.int32(H * N)

    in_maps = []
    for c in range(NCORES):
        l0 = c * LC
        lo = l0 - HALO
        lo_clip = max(lo, 0)
        nvalid = (l0 + LC) - lo_clip
        r0 = (lo_clip - lo) * B
        ids_c = np.full((MP, H), OOB, np.int32)
        seg = hash_input_ids[lo_clip : l0 + LC].reshape(nvalid * B, H)
        ids_c[r0 : r0 + nvalid * B] = (seg.astype(np.int64) + head_off).astype(
            np.int32
        )
        hid_c = np.zeros((MP, D), bf)
        hseg = hidden_states[lo_clip : l0 + LC].reshape(nvalid * B, D)
        hid_c[r0 : r0 + nvalid * B] = hseg.astype(bf)
        ids_r = np.ascontiguousarray(
            ids_c.reshape(MT, 128, H).transpose(1, 0, 2).reshape(128, MT * H)
        )
        in_maps.append(
            {
                "tabs": tabs_np,
                "ids": ids_r,
                "hid": hid_c,
                "wkv": wkv_np,
                "scal": scal_np,
            }
        )
    return in_maps


def unshard_output(results):
    """results: list of per-core dicts with 'outT' [D, MOUT] -> [L, B, D]."""
    out = np.empty((L, B, D), np.float32)
    for c in range(NCORES):
        o = np.asarray(results[c]["outT"], dtype=np.float32)
        out[c * LC : (c + 1) * LC] = o.reshape(D, LC, B).transpose(1, 2, 0)
    return out


def kernel(hidden_states, hash_input_ids, emb_tables, key_w, key_b,
           norm1_w, norm2_w, value_w, value_b, conv_w, conv_b):
    args = [hidden_states, hash_input_ids, emb_tables, key_w, key_b,
            norm1_w, norm2_w, value_w, value_b, conv_w, conv_b]
    args = [np.asarray(a) for a in args]
    in_maps = host_prep(*args)
    nc = _get_program()
    res = run_bass_kernel_spmd(nc, in_maps, list(range(NCORES)))
    return unshard_output(res.results)
